# revision 23
# baseline (speedup 1.0000x reference)
"""Trainium2 Bass kernel for the Centroid (segment_reduce) problem.

new_centroid = 0.3 * (segment_sum(embed, y) / counts) + 0.7 * centroid
  embed [32768, 1024] f32, y [32768] int64 (0..999), centroid [1000, 1024] f32

Strategy (8 NeuronCores, CLASS-sharded via host-side partitioning):
  - The host knows y, so it partitions classes into 8 bins of <=128
    classes balanced to ~4096 rows each (greedy + swap repair); core i
    receives ALL rows of its classes -> full local sums, NO collectives.
  - Scatter-add as a one-hot matmul on TensorE (fp8 DoubleRow) with a
    single 128-class M-tile (labels in-range by construction): 8x less
    PE work than batch-parallel sharding.
  - embed lives in DRAM partition-major [P, KT, D] so the shard moves in
    ~10 large DMAs split over both HWDGE rings (sync/scalar).
  - The first 4 row-tiles are laid out identity-style (row slot p holds
    a row of local class p), so their one-hot is the data-independent
    identity pattern: the first 4 matmuls depend only on their embed
    DMA, not on the y transfer.
  - Remaining one-hots: one DVE is_equal per DMA group using stride-0
    broadcast APs (iota vs labels), fp8 output.
  - Counts are known on the host (bincount): 0.3/count and 0.7*centroid
    ride one bf16 aux tensor; finalize is one fused scalar_tensor_tensor
    (psum*recip + cent) per 512-col PSUM chunk.
  - Padding rows carry label -1 -> all-zero one-hot -> contribute 0.
  - Output is bf16 [128, 1024] per core; host casts/assembles to f32.
"""

import numpy as np

import concourse.bacc as bacc
import concourse.mybir as mybir
import concourse.tile as tile
from concourse.bass_utils import run_bass_kernel_spmd

N_CORES = 8
C = 1000  # classes
D = 1024  # embed dim
B = 32768  # total batch
P = 128
KT = 33  # 128-row tiles of capacity per core (16 DoubleRow pairs + 1)
CAP = KT * P  # 4224 row slots (4096 rows + identity-block waste + slack)
ID_TILES = 4  # leading tiles in identity layout (slot p -> local class p)
FACTOR = 0.3
NCHUNK = 2  # PSUM passes: 2 x 512 f32 cols (one bank each)
CW = D // NCHUNK  # 512
# embed DMA groups in TILES. Each HWDGE ring caps at ~205 GB/s (2KB
# descriptor generation), and early in the window the sync ring gets a
# ~4x larger share of the fabric than the scalar ring. So: bias the
# early tiles onto sync, and emit matmuls in expected-ARRIVAL order so
# the PE never stalls on a late ring while data from the other sits
# ready. (ring, tiles); consumption follows list order; tile ranges are
# assigned in this order too.
# 8 groups exactly (plus y+aux = 10 DMAs pre-output: more would hit the
# scheduler's 8 completion-sem lanes and gate late issues). Sizes follow
# the measured ring rates: sync ~210 GB/s from the start, scalar crawls
# for ~3.5us then runs ~240 GB/s, so sync carries the early tiles and
# scalar's groups slot into later consumption positions. The last group
# is small (1 pair + the odd tile) to shorten the post-receipt tail.
# Measured: the scalar ring's completion sems fire ~3.5-4.4us after its
# last byte (sync: ~0.6us), so sync carries more bytes, the tail groups,
# and the output stores; scalar only mid-window groups whose lag hides
# behind later consumption.
GROUPS = [
    ("s", 2),  # identity tiles 0-1
    ("s", 2),  # identity tiles 2-3
    ("s", 4),
    ("s", 4),
    ("c", 6),
    ("s", 4),
    ("s", 2),
    ("c", 6),
    ("s", 2),
    ("c", 1),  # lone odd tile last: post-receipt tail is 2 short matmuls
]
assert sum(n for _, n in GROUPS) == KT
AUXW = 1032  # bf16 aux row: [0]=0.3/count, [1:1+D]=0.7*centroid, pad

_F32 = mybir.dt.float32
_BF16 = mybir.dt.bfloat16
_FP8 = mybir.dt.float8e4

_CACHE: dict = {}


def _build():
    nc = bacc.Bacc(
        "TRN2", target_bir_lowering=False, debug=False, num_devices=N_CORES
    )
    emb_d = nc.dram_tensor("emb", [P, KT, D], _FP8, kind="ExternalInput").ap()
    yt_d = nc.dram_tensor("yt", [P, KT], _BF16, kind="ExternalInput").ap()
    aux_d = nc.dram_tensor("aux", [P, AUXW], _BF16, kind="ExternalInput").ap()
    out_d = nc.dram_tensor("out", [P, D], _BF16, kind="ExternalOutput").ap()

    with tile.TileContext(nc) as tc:
        with (
            tc.tile_pool(name="const", bufs=1) as const_pool,
            tc.tile_pool(name="emb", bufs=len(GROUPS)) as emb_pool,
            tc.tile_pool(name="oh", bufs=len(GROUPS)) as oh_pool,
            tc.tile_pool(name="psum", bufs=NCHUNK, space="PSUM") as psum_pool,
            tc.tile_pool(name="fin", bufs=2) as fin_pool,
        ):
            # iota row (iota_r[p, c] = c) and iota col (iota_c[p, 0] = p)
            iota_r = const_pool.tile([P, P], _BF16)
            nc.gpsimd.iota(
                iota_r[:],
                pattern=[[1, P]],
                base=0,
                channel_multiplier=0,
                allow_small_or_imprecise_dtypes=True,
            )
            iota_c = const_pool.tile([P, 1], _F32)
            nc.gpsimd.iota(
                iota_c[:],
                pattern=[[0, 1]],
                base=0,
                channel_multiplier=1,
                allow_small_or_imprecise_dtypes=True,
            )
            # identity one-hot (shared by the first ID_TILES tiles):
            # oh_id[p, kk, c] = (c == p)
            oh_id = const_pool.tile([P, 2, P], _FP8)
            nc.vector.tensor_scalar(
                oh_id[:],
                iota_r[:].unsqueeze(1).broadcast_to([P, 2, P]),
                iota_c[:, 0:1],
                None,
                mybir.AluOpType.is_equal,
            )

            # tiny y transfer first on the sync ring (its completion sem
            # fires promptly), then the embed groups in list order
            y_all = const_pool.tile([P, KT], _BF16)
            nc.sync.dma_start(out=y_all[:], in_=yt_d[:])
            emb_tiles = []
            t0 = 0
            for g, (ring, nt) in enumerate(GROUPS):
                emb_t = emb_pool.tile([P, nt, D], _FP8, name=f"emb{g}", tag="emb")
                dma_eng = nc.sync if ring == "s" else nc.scalar
                dma_eng.dma_start(out=emb_t[:], in_=emb_d[:, t0 : t0 + nt])
                emb_tiles.append((emb_t, t0, nt))
                t0 += nt

            # aux constants late on the scalar ring (needed only at finalize)
            aux = const_pool.tile([P, AUXW], _BF16)
            nc.scalar.dma_start(out=aux[:], in_=aux_d[:])

            # one is_equal per non-identity group:
            # oh[p, kk, c] = (iota_r[c] == y[p, t0+kk])
            oh_tiles = []
            for g, (emb_t, t0, nt) in enumerate(emb_tiles):
                if t0 + nt <= ID_TILES:
                    oh_tiles.append(None)
                    continue
                oh_t = oh_pool.tile([P, nt, P], _FP8, name=f"oh{g}", tag="oh")
                io_b = iota_r[:].unsqueeze(1).broadcast_to([P, nt, P])
                y_b = y_all[:, t0 : t0 + nt].unsqueeze(2).broadcast_to([P, nt, P])
                nc.vector.tensor_tensor(
                    out=oh_t[:], in0=io_b, in1=y_b, op=mybir.AluOpType.is_equal
                )
                oh_tiles.append(oh_t)

            psums = [
                psum_pool.tile([P, CW], _F32, name=f"ps{c}", tag="ps")
                for c in range(NCHUNK)
            ]

            # dummy matmuls on the already-resident iota tile: keep the PE
            # busy from ~t=8us so the HAM clock gate is warm (2.4 GHz) by
            # the time real data lands; they only depend on the iota op
            ps_warm = psum_pool.tile([P, P], _F32, name="ps_warm", tag="psw")
            for w in range(24):
                nc.tensor.matmul(
                    ps_warm[:],
                    lhsT=iota_r[:],
                    rhs=iota_r[:],
                    start=True,
                    stop=True,
                )

            def mm(lhsT, rhs_pair, first, last, cidx, dr):
                nc.tensor.matmul(
                    psums[cidx][:],
                    lhsT=lhsT,
                    rhs=rhs_pair[:, :, cidx * CW : (cidx + 1) * CW]
                    if dr
                    else rhs_pair[:, cidx * CW : (cidx + 1) * CW],
                    start=first,
                    stop=last,
                    perf_mode=mybir.MatmulPerfMode.DoubleRow if dr else None,
                )

            # j-major over tiles in DMA-arrival order; DoubleRow pairs,
            # odd tail tile as a plain matmul (FWL path)
            for g, (emb_t, t0, nt) in enumerate(emb_tiles):
                kk = 0
                while kk < nt:
                    k = t0 + kk
                    first = k == 0
                    if nt - kk >= 2:
                        last = k + 2 == KT
                        if k + 2 <= ID_TILES:
                            lhsT = oh_id[:]
                        else:
                            lhsT = oh_tiles[g][:, kk : kk + 2, :]
                        rhs = emb_t[:, kk : kk + 2, :]
                        # on the very last tiles, finish chunk 1 first so
                        # its finalize overlaps chunk 0's last matmul
                        order = (1, 0) if last else (0, 1)
                        for cidx in order:
                            mm(lhsT, rhs, first, last, cidx, dr=True)
                        kk += 2
                    else:
                        last = k + 1 == KT
                        lhsT = oh_tiles[g][:, kk, :]
                        rhs = emb_t[:, kk, :]
                        order = (1, 0) if last else (0, 1)
                        for cidx in order:
                            mm(lhsT, rhs, first, last, cidx, dr=False)
                        kk += 1

            # fused finalize: out = (0.3/count) * sums + 0.7*centroid
            for cidx, eng in ((1, nc.vector), (0, nc.vector)):
                cols = slice(cidx * CW, (cidx + 1) * CW)
                out_sb = fin_pool.tile([P, CW], _BF16, name=f"o{cidx}", tag="o")
                eng.scalar_tensor_tensor(
                    out=out_sb[:],
                    in0=psums[cidx][:],
                    scalar=aux[:, 0:1],
                    in1=aux[:, 1 + cidx * CW : 1 + (cidx + 1) * CW],
                    op0=mybir.AluOpType.mult,
                    op1=mybir.AluOpType.add,
                )
                # both output stores on the sync ring: its completion sem
                # fires ~0.6us after last byte vs scalar's ~4us
                nc.sync.dma_start(out=out_d[:, cols], in_=out_sb[:])

    nc.compile()
    return nc


def get_nc():
    if "nc" not in _CACHE:
        _CACHE["nc"] = _build()
    return _CACHE["nc"]


def _balance(counts: np.ndarray) -> list[list[int]]:
    """Partition classes into 8 bins, <=128 classes, ~B/8 rows each."""
    target = int(counts.sum()) // N_CORES
    order = np.argsort(-counts)
    bins: list[list[int]] = [[] for _ in range(N_CORES)]
    loads = np.zeros(N_CORES, dtype=np.int64)
    for c in order:
        eligible = [b for b in range(N_CORES) if len(bins[b]) < P]
        b = min(eligible, key=lambda i: loads[i])
        bins[b].append(int(c))
        loads[b] += counts[c]
    for _ in range(4 * C):
        hi = int(np.argmax(loads))
        lo = int(np.argmin(loads))
        diff = int(loads[hi] - loads[lo])
        if diff <= 0:
            break
        best = None
        for a in bins[hi]:
            for bb in bins[lo]:
                delta = int(counts[a] - counts[bb])
                if 0 < delta <= diff:
                    score = abs(delta - diff / 2)
                    if best is None or score < best[0]:
                        best = (score, a, bb)
        if best is None:
            break
        _, a, bb = best
        bins[hi].remove(a)
        bins[lo].remove(bb)
        bins[hi].append(bb)
        bins[lo].append(a)
        loads[hi] += counts[bb] - counts[a]
        loads[lo] += counts[a] - counts[bb]
    return bins


def make_in_maps(embed: np.ndarray, y: np.ndarray, centroid: np.ndarray):
    fp8_np = mybir.dt.np(_FP8)
    bf16_np = mybir.dt.np(_BF16)
    embed8 = np.ascontiguousarray(embed, dtype=np.float32).astype(fp8_np)
    y_i = np.ascontiguousarray(y).astype(np.int64)
    cent = np.asarray(centroid, dtype=np.float32)
    counts = np.bincount(y_i, minlength=C).astype(np.int64)
    bins = _balance(counts)

    in_maps = []
    meta = []
    for i in range(N_CORES):
        cls = np.array(sorted(bins[i]), dtype=np.int64)
        ncls = len(cls)
        assert 0 < ncls <= P
        # local index lookup: class value -> 0..ncls-1 (else -1)
        lut = np.full(C, -1, dtype=np.int64)
        lut[cls] = np.arange(ncls)
        loc_all = lut[y_i]
        rows = np.flatnonzero(loc_all >= 0)
        loc = loc_all[rows]
        n = len(rows)
        # order rows by class; position within class decides placement
        srt = np.argsort(loc, kind="stable")
        rows, loc = rows[srt], loc[srt]
        bnd = np.searchsorted(loc, np.arange(ncls))
        pos = np.arange(n) - bnd[loc]
        assert counts[cls].min() >= ID_TILES, "identity block needs >=4 rows"
        id_mask = pos < ID_TILES
        slot = np.where(id_mask, pos * P + loc, 0)
        n_rest = int(n - id_mask.sum())
        assert ID_TILES * P + n_rest <= CAP, f"core {i}: {n} rows overflow"
        slot[~id_mask] = ID_TILES * P + np.arange(n_rest)
        emb_buf = np.zeros((CAP, D), dtype=fp8_np)
        emb_buf[slot] = embed8[rows]
        ylab = np.full(CAP, -1.0, dtype=np.float32)
        ylab[slot] = loc.astype(np.float32)
        # slot = k*128 + p lives at [p, k]
        emb_buf = np.ascontiguousarray(
            emb_buf.reshape(KT, P, D).transpose(1, 0, 2)
        )
        yt = np.ascontiguousarray(ylab.reshape(KT, P).T).astype(bf16_np)
        aux = np.zeros((P, AUXW), dtype=np.float32)
        aux[:ncls, 0] = FACTOR / counts[cls]
        aux[:ncls, 1 : 1 + D] = (1.0 - FACTOR) * cent[cls]
        in_maps.append({"emb": emb_buf, "yt": yt, "aux": aux.astype(bf16_np)})
        meta.append(cls)
    return in_maps, meta


def kernel(embed: np.ndarray, y: np.ndarray, centroid: np.ndarray) -> np.ndarray:
    nc = get_nc()
    in_maps, meta = make_in_maps(embed, y, centroid)
    res = run_bass_kernel_spmd(nc, in_maps, core_ids=list(range(N_CORES)))
    full = np.empty((C, D), dtype=np.float32)
    for i, cls in enumerate(meta):
        full[cls] = res.results[i]["out"][: len(cls)].astype(np.float32)
    return full


# revision 24
# speedup vs baseline: 1.0816x; 1.0816x over previous
"""Trainium2 Bass kernel for the Centroid (segment_reduce) problem.

new_centroid = 0.3 * (segment_sum(embed, y) / counts) + 0.7 * centroid
  embed [32768, 1024] f32, y [32768] int64 (0..999), centroid [1000, 1024] f32

Strategy (8 NeuronCores, CLASS-sharded via host-side partitioning):
  - The host knows y, so it partitions classes into 8 bins of <=128
    classes balanced to ~4096 rows each (greedy + swap repair); core i
    receives ALL rows of its classes -> full local sums, NO collectives.
  - Scatter-add as a one-hot matmul on TensorE (fp8 DoubleRow) with a
    single 128-class M-tile (labels in-range by construction): 8x less
    PE work than batch-parallel sharding.
  - embed lives in DRAM partition-major [P, KT, D] so the shard moves in
    ~10 large DMAs split over both HWDGE rings (sync/scalar).
  - The first 4 row-tiles are laid out identity-style (row slot p holds
    a row of local class p), so their one-hot is the data-independent
    identity pattern: the first 4 matmuls depend only on their embed
    DMA, not on the y transfer.
  - Remaining one-hots: one DVE is_equal per DMA group using stride-0
    broadcast APs (iota vs labels), fp8 output.
  - Counts are known on the host (bincount): 0.3/count and 0.7*centroid
    ride one bf16 aux tensor; finalize is one fused scalar_tensor_tensor
    (psum*recip + cent) per 512-col PSUM chunk.
  - Padding rows carry label -1 -> all-zero one-hot -> contribute 0.
  - Output is bf16 [128, 1024] per core; host casts/assembles to f32.
"""

import numpy as np

import concourse.bacc as bacc
import concourse.mybir as mybir
import concourse.tile as tile
from concourse.bass_utils import run_bass_kernel_spmd

N_CORES = 8
C = 1000  # classes
D = 1024  # embed dim
B = 32768  # total batch
P = 128
KT = 33  # 128-row tiles of capacity per core (16 DoubleRow pairs + 1)
CAP = KT * P  # 4224 row slots (4096 rows + identity-block waste + slack)
ID_TILES = 4  # leading tiles in identity layout (slot p -> local class p)
FACTOR = 0.3
NCHUNK = 2  # PSUM passes: 2 x 512 f32 cols (one bank each)
CW = D // NCHUNK  # 512
# embed DMA groups in TILES. Each HWDGE ring caps at ~205 GB/s (2KB
# descriptor generation), and early in the window the sync ring gets a
# ~4x larger share of the fabric than the scalar ring. So: bias the
# early tiles onto sync, and emit matmuls in expected-ARRIVAL order so
# the PE never stalls on a late ring while data from the other sits
# ready. (ring, tiles); consumption follows list order; tile ranges are
# assigned in this order too.
# 8 groups exactly (plus y+aux = 10 DMAs pre-output: more would hit the
# scheduler's 8 completion-sem lanes and gate late issues). Sizes follow
# the measured ring rates: sync ~210 GB/s from the start, scalar crawls
# for ~3.5us then runs ~240 GB/s, so sync carries the early tiles and
# scalar's groups slot into later consumption positions. The last group
# is small (1 pair + the odd tile) to shorten the post-receipt tail.
# Measured: the scalar ring's completion sems fire ~3.5-4.4us after its
# last byte (sync: ~0.6us), so sync carries more bytes, the tail groups,
# and the output stores; scalar only mid-window groups whose lag hides
# behind later consumption.
GROUPS = [
    ("s", 2),  # identity tiles 0-1
    ("s", 2),  # identity tiles 2-3
    ("c", 4),
    ("s", 4),
    ("c", 4),
    ("s", 4),
    ("c", 5),
    ("s", 4),
    ("s", 2),
    ("s", 2),
]
assert sum(n for _, n in GROUPS) == KT
AUXW = 1032  # bf16 aux row: [0]=0.3/count, [1:1+D]=0.7*centroid, pad

_F32 = mybir.dt.float32
_BF16 = mybir.dt.bfloat16
_FP8 = mybir.dt.float8e4

_CACHE: dict = {}


def _build():
    nc = bacc.Bacc(
        "TRN2", target_bir_lowering=False, debug=False, num_devices=N_CORES
    )
    emb_d = nc.dram_tensor("emb", [P, KT, D], _FP8, kind="ExternalInput").ap()
    yt_d = nc.dram_tensor("yt", [P, KT], _BF16, kind="ExternalInput").ap()
    aux_d = nc.dram_tensor("aux", [P, AUXW], _BF16, kind="ExternalInput").ap()
    out_d = nc.dram_tensor("out", [P, D], _BF16, kind="ExternalOutput").ap()

    with tile.TileContext(nc) as tc:
        with (
            tc.tile_pool(name="const", bufs=1) as const_pool,
            tc.tile_pool(name="emb", bufs=len(GROUPS)) as emb_pool,
            tc.tile_pool(name="oh", bufs=len(GROUPS)) as oh_pool,
            tc.tile_pool(name="psum", bufs=NCHUNK, space="PSUM") as psum_pool,
            tc.tile_pool(name="fin", bufs=2) as fin_pool,
        ):
            # iota row (iota_r[p, c] = c) and iota col (iota_c[p, 0] = p)
            iota_r = const_pool.tile([P, P], _BF16)
            nc.gpsimd.iota(
                iota_r[:],
                pattern=[[1, P]],
                base=0,
                channel_multiplier=0,
                allow_small_or_imprecise_dtypes=True,
            )
            iota_c = const_pool.tile([P, 1], _F32)
            nc.gpsimd.iota(
                iota_c[:],
                pattern=[[0, 1]],
                base=0,
                channel_multiplier=1,
                allow_small_or_imprecise_dtypes=True,
            )
            # identity one-hot (shared by the first ID_TILES tiles):
            # oh_id[p, kk, c] = (c == p)
            oh_id = const_pool.tile([P, 2, P], _FP8)
            nc.vector.tensor_scalar(
                oh_id[:],
                iota_r[:].unsqueeze(1).broadcast_to([P, 2, P]),
                iota_c[:, 0:1],
                None,
                mybir.AluOpType.is_equal,
            )

            # tiny y transfer first on the sync ring (its completion sem
            # fires promptly), then the embed groups in list order
            y_all = const_pool.tile([P, KT], _BF16)
            nc.sync.dma_start(out=y_all[:], in_=yt_d[:])
            emb_tiles = []
            t0 = 0
            for g, (ring, nt) in enumerate(GROUPS):
                emb_t = emb_pool.tile([P, nt, D], _FP8, name=f"emb{g}", tag="emb")
                dma_eng = nc.sync if ring == "s" else nc.scalar
                dma_eng.dma_start(out=emb_t[:], in_=emb_d[:, t0 : t0 + nt])
                emb_tiles.append((emb_t, t0, nt))
                t0 += nt

            # aux constants late on the scalar ring (needed only at finalize)
            aux = const_pool.tile([P, AUXW], _BF16)
            nc.scalar.dma_start(out=aux[:], in_=aux_d[:])

            # one is_equal per non-identity group:
            # oh[p, kk, c] = (iota_r[c] == y[p, t0+kk])
            oh_tiles = []
            for g, (emb_t, t0, nt) in enumerate(emb_tiles):
                if t0 + nt <= ID_TILES:
                    oh_tiles.append(None)
                    continue
                oh_t = oh_pool.tile([P, nt, P], _FP8, name=f"oh{g}", tag="oh")
                io_b = iota_r[:].unsqueeze(1).broadcast_to([P, nt, P])
                y_b = y_all[:, t0 : t0 + nt].unsqueeze(2).broadcast_to([P, nt, P])
                nc.vector.tensor_tensor(
                    out=oh_t[:], in0=io_b, in1=y_b, op=mybir.AluOpType.is_equal
                )
                oh_tiles.append(oh_t)

            psums = [
                psum_pool.tile([P, CW], _F32, name=f"ps{c}", tag="ps")
                for c in range(NCHUNK)
            ]

            # dummy matmuls on the already-resident iota tile: keep the PE
            # busy from ~t=8us so the HAM clock gate is warm (2.4 GHz) by
            # the time real data lands; they only depend on the iota op
            ps_warm = psum_pool.tile([P, P], _F32, name="ps_warm", tag="psw")
            for w in range(24):
                nc.tensor.matmul(
                    ps_warm[:],
                    lhsT=iota_r[:],
                    rhs=iota_r[:],
                    start=True,
                    stop=True,
                )

            def mm(lhsT, rhs_pair, first, last, cidx, dr):
                nc.tensor.matmul(
                    psums[cidx][:],
                    lhsT=lhsT,
                    rhs=rhs_pair[:, :, cidx * CW : (cidx + 1) * CW]
                    if dr
                    else rhs_pair[:, cidx * CW : (cidx + 1) * CW],
                    start=first,
                    stop=last,
                    perf_mode=mybir.MatmulPerfMode.DoubleRow if dr else None,
                )

            # j-major over tiles in DMA-arrival order; DoubleRow pairs,
            # odd tail tile as a plain matmul (FWL path)
            for g, (emb_t, t0, nt) in enumerate(emb_tiles):
                kk = 0
                while kk < nt:
                    k = t0 + kk
                    first = k == 0
                    if nt - kk >= 2:
                        last = k + 2 == KT
                        if k + 2 <= ID_TILES:
                            lhsT = oh_id[:]
                        else:
                            lhsT = oh_tiles[g][:, kk : kk + 2, :]
                        rhs = emb_t[:, kk : kk + 2, :]
                        # on the very last tiles, finish chunk 1 first so
                        # its finalize overlaps chunk 0's last matmul
                        order = (1, 0) if last else (0, 1)
                        for cidx in order:
                            mm(lhsT, rhs, first, last, cidx, dr=True)
                        kk += 2
                    else:
                        last = k + 1 == KT
                        lhsT = oh_tiles[g][:, kk, :]
                        rhs = emb_t[:, kk, :]
                        order = (1, 0) if last else (0, 1)
                        for cidx in order:
                            mm(lhsT, rhs, first, last, cidx, dr=False)
                        kk += 1

            # fused finalize: out = (0.3/count) * sums + 0.7*centroid
            for cidx, eng in ((1, nc.vector), (0, nc.vector)):
                cols = slice(cidx * CW, (cidx + 1) * CW)
                out_sb = fin_pool.tile([P, CW], _BF16, name=f"o{cidx}", tag="o")
                eng.scalar_tensor_tensor(
                    out=out_sb[:],
                    in0=psums[cidx][:],
                    scalar=aux[:, 0:1],
                    in1=aux[:, 1 + cidx * CW : 1 + (cidx + 1) * CW],
                    op0=mybir.AluOpType.mult,
                    op1=mybir.AluOpType.add,
                )
                # both output stores on the sync ring: its completion sem
                # fires ~0.6us after last byte vs scalar's ~4us
                nc.sync.dma_start(out=out_d[:, cols], in_=out_sb[:])

    nc.compile()
    return nc


def get_nc():
    if "nc" not in _CACHE:
        _CACHE["nc"] = _build()
    return _CACHE["nc"]


def _balance(counts: np.ndarray) -> list[list[int]]:
    """Partition classes into 8 bins, <=128 classes, ~B/8 rows each."""
    target = int(counts.sum()) // N_CORES
    order = np.argsort(-counts)
    bins: list[list[int]] = [[] for _ in range(N_CORES)]
    loads = np.zeros(N_CORES, dtype=np.int64)
    for c in order:
        eligible = [b for b in range(N_CORES) if len(bins[b]) < P]
        b = min(eligible, key=lambda i: loads[i])
        bins[b].append(int(c))
        loads[b] += counts[c]
    for _ in range(4 * C):
        hi = int(np.argmax(loads))
        lo = int(np.argmin(loads))
        diff = int(loads[hi] - loads[lo])
        if diff <= 0:
            break
        best = None
        for a in bins[hi]:
            for bb in bins[lo]:
                delta = int(counts[a] - counts[bb])
                if 0 < delta <= diff:
                    score = abs(delta - diff / 2)
                    if best is None or score < best[0]:
                        best = (score, a, bb)
        if best is None:
            break
        _, a, bb = best
        bins[hi].remove(a)
        bins[lo].remove(bb)
        bins[hi].append(bb)
        bins[lo].append(a)
        loads[hi] += counts[bb] - counts[a]
        loads[lo] += counts[a] - counts[bb]
    return bins


def make_in_maps(embed: np.ndarray, y: np.ndarray, centroid: np.ndarray):
    fp8_np = mybir.dt.np(_FP8)
    bf16_np = mybir.dt.np(_BF16)
    embed8 = np.ascontiguousarray(embed, dtype=np.float32).astype(fp8_np)
    y_i = np.ascontiguousarray(y).astype(np.int64)
    cent = np.asarray(centroid, dtype=np.float32)
    counts = np.bincount(y_i, minlength=C).astype(np.int64)
    bins = _balance(counts)

    in_maps = []
    meta = []
    for i in range(N_CORES):
        cls = np.array(sorted(bins[i]), dtype=np.int64)
        ncls = len(cls)
        assert 0 < ncls <= P
        # local index lookup: class value -> 0..ncls-1 (else -1)
        lut = np.full(C, -1, dtype=np.int64)
        lut[cls] = np.arange(ncls)
        loc_all = lut[y_i]
        rows = np.flatnonzero(loc_all >= 0)
        loc = loc_all[rows]
        n = len(rows)
        # order rows by class; position within class decides placement
        srt = np.argsort(loc, kind="stable")
        rows, loc = rows[srt], loc[srt]
        bnd = np.searchsorted(loc, np.arange(ncls))
        pos = np.arange(n) - bnd[loc]
        assert counts[cls].min() >= ID_TILES, "identity block needs >=4 rows"
        id_mask = pos < ID_TILES
        slot = np.where(id_mask, pos * P + loc, 0)
        n_rest = int(n - id_mask.sum())
        assert ID_TILES * P + n_rest <= CAP, f"core {i}: {n} rows overflow"
        slot[~id_mask] = ID_TILES * P + np.arange(n_rest)
        emb_buf = np.zeros((CAP, D), dtype=fp8_np)
        emb_buf[slot] = embed8[rows]
        ylab = np.full(CAP, -1.0, dtype=np.float32)
        ylab[slot] = loc.astype(np.float32)
        # slot = k*128 + p lives at [p, k]
        emb_buf = np.ascontiguousarray(
            emb_buf.reshape(KT, P, D).transpose(1, 0, 2)
        )
        yt = np.ascontiguousarray(ylab.reshape(KT, P).T).astype(bf16_np)
        aux = np.zeros((P, AUXW), dtype=np.float32)
        aux[:ncls, 0] = FACTOR / counts[cls]
        aux[:ncls, 1 : 1 + D] = (1.0 - FACTOR) * cent[cls]
        in_maps.append({"emb": emb_buf, "yt": yt, "aux": aux.astype(bf16_np)})
        meta.append(cls)
    return in_maps, meta


def kernel(embed: np.ndarray, y: np.ndarray, centroid: np.ndarray) -> np.ndarray:
    nc = get_nc()
    in_maps, meta = make_in_maps(embed, y, centroid)
    res = run_bass_kernel_spmd(nc, in_maps, core_ids=list(range(N_CORES)))
    full = np.empty((C, D), dtype=np.float32)
    for i, cls in enumerate(meta):
        full[cls] = res.results[i]["out"][: len(cls)].astype(np.float32)
    return full
